# revision 1
# baseline (speedup 1.0000x reference)
"""Trainium2 Bass kernel for CapsuleLayer dynamic routing (8-core SPMD).

Strategy: shard the 2048 input capsules (n) across 8 cores. Each core builds
u_hat = einsum('bni,nio->bno') for its n-slice with W as the PE-stationary
operand so the PSUM output lands with (i4,j32) on partitions -- the native
layout for the routing b-update contraction over j. The o-contraction over n
runs on an n-partition view produced by DMA xbar transposes. Routing's
per-iteration global sum over n is an AllReduce of the tiny [32,32,32] o.
"""
import sys

sys.path.insert(0, "/opt/trn_rl_repo")

import numpy as np
import ml_dtypes

B = 32            # batch
N_TOTAL = 2048    # input capsules
KD = 16           # input capsule dim
NCAP = 32         # output capsules (i)
DIM = 32          # output capsule dim (j)
O = NCAP * DIM    # 1024
NUM_CORES = 8
NL = N_TOTAL // NUM_CORES   # 256 local n
G = NL // 8                 # 32 groups of 8 n
OC = O // 128               # 8 o-chunks
EPS_K = 1e-7
EPS_L2 = 1e-12

_PROG = {}


def _set_dims(ap, dims, offset=None):
    c = ap.copy()
    v = c.ap
    while len(v) > len(dims):
        v.pop()
    while len(v) < len(dims):
        v.insert(0, [0, 1])
    for k, d in enumerate(dims):
        v[k] = list(d)
    if offset is not None:
        c.offset = offset
    return c


def build_program(num_cores):
    import concourse.bass as bass
    import concourse.mybir as mybir
    from concourse import bacc, tile
    from concourse.tile import add_dep_helper

    f32 = mybir.dt.float32
    bf16 = mybir.dt.bfloat16
    AX = mybir.AxisListType
    OP = mybir.AluOpType
    AF = mybir.ActivationFunctionType

    nc = bacc.Bacc("TRN2", target_bir_lowering=False, num_devices=num_cores)
    rg = [list(range(num_cores))]

    wl_d = nc.dram_tensor("wl", [G, OC, 128, 128], bf16, kind="ExternalInput")
    ubd_d = nc.dram_tensor("ubd", [G, 128, 256], bf16, kind="ExternalInput")
    upl_d = nc.dram_tensor("upl", [G, 128, B], bf16, kind="ExternalInput")
    out_d = nc.dram_tensor("out", [B, O], f32, kind="ExternalOutput")

    with tile.TileContext(nc) as tc:
        with (
            tc.tile_pool(name="pers", bufs=1) as pers,
            tc.tile_pool(name="dram", bufs=1, space="DRAM") as dram,
            tc.tile_pool(name="ps_main", bufs=2, space="PSUM") as ps_main,
            tc.tile_pool(name="small", bufs=1) as small,
        ):
            u_hat = pers.tile([128, B, OC, NL], bf16, tag="u_hat")
            o_acc = pers.tile([DIM, B, NCAP], f32, tag="o_acc")
            tr_scr = pers.tile([128, 1024], f32, tag="tr_scr")
            o0_sb = pers.tile([128, OC, B], f32, tag="o0_sb")

            onrm = small.tile([B, O], f32, tag="onrm")
            onl = small.tile([B, O], f32, tag="onl")
            s2 = small.tile([B, NCAP], f32, tag="s2")
            s2b = small.tile([B, NCAP], f32, tag="s2b")
            s2c = small.tile([B, NCAP], f32, tag="s2c")
            rinv = small.tile([B, NCAP], f32, tag="rinv")
            mx = small.tile([128, 2 * B], f32, tag="mx")
            sm = small.tile([128, 2 * B], f32, tag="sm")
            smr = small.tile([128, 2 * B], f32, tag="smr")

            cc_in = [
                dram.tile([B, O], f32, tag=f"cc_in{t}", name=f"cc_in{t}")
                for t in range(3)
            ]
            cc_out = [
                dram.tile([B, O], f32, tag=f"cc_out{t}", name=f"cc_out{t}")
                for t in range(3)
            ]
            o_dram = dram.tile([B, O], f32, tag="o_dram", name="o_dram")

            # ---------------- Phase A: load + build u_hat + o0 ----------------
            with (
                tc.tile_pool(name="bpool", bufs=1) as bpool,
                tc.tile_pool(name="wpool", bufs=2) as wpool,
                tc.tile_pool(name="ps_o0", bufs=1, space="PSUM") as ps_o0,
            ):
                ubd_sb = bpool.tile([128, G, 256], bf16, tag="ubd_sb")
                upl_sb = bpool.tile([128, G, B], bf16, tag="upl_sb")
                nc.sync.dma_start(ubd_sb[:], ubd_d[:].rearrange("g p c -> p g c"))
                nc.sync.dma_start(upl_sb[:], upl_d[:].rearrange("g p c -> p g c"))

                for oc in range(OC):
                    wl_oc = wpool.tile([128, G, 128], bf16, tag="wl_oc")
                    nc.sync.dma_start(
                        wl_oc[:], wl_d[:, oc, :, :].rearrange("g p c -> p g c")
                    )
                    o0ps = ps_o0.tile([128, B], f32, tag="o0ps")
                    for g4 in range(G // 4):
                        ps = ps_main.tile([128, 1024], f32, tag="ps")
                        for gh in range(4):
                            g = g4 * 4 + gh
                            nc.tensor.matmul(
                                ps[:, gh * 256:(gh + 1) * 256],
                                wl_oc[:, g, :], ubd_sb[:, g, :],
                                start=True, stop=True,
                                skip_group_check=True,
                            )
                            nc.tensor.matmul(
                                o0ps[:], wl_oc[:, g, :], upl_sb[:, g, :],
                                start=(g == 0), stop=(g == G - 1),
                                skip_group_check=True,
                            )
                        # psum cols (gh, b, n8) -> u_hat[:, :, oc, g4*32:+32]
                        g = g4 * 4
                        dst = _set_dims(
                            u_hat[:, :, oc, 0],
                            [[B * OC * NL, 128], [8, 4], [OC * NL, B], [1, 8]],
                            offset=oc * NL + g * 8,
                        )
                        src = ps[:].rearrange("p (h b n) -> p h b n", h=4, b=B)
                        if g4 % 2 == 0:
                            nc.scalar.copy(dst, src)
                        else:
                            nc.vector.tensor_copy(dst, src)
                    nc.scalar.copy(o0_sb[:, oc, :], o0ps[:])

            # o0 partial -> dram bounce: cc_in0[b, oc*128+p] = o0_sb[p, oc, b]
            for oc in range(OC):
                src = o0_sb[:, oc, :]
                dst = _set_dims(
                    cc_in[0][:], [[1, 128], [O, B]], offset=oc * 128
                )
                nc.sync.dma_start(dst, src)

            def all_reduce(t):
                if num_cores == 1:
                    nc.gpsimd.dma_start(cc_out[t][:], cc_in[t][:])
                else:
                    nc.gpsimd.collective_compute(
                        "AllReduce", OP.add, replica_groups=rg,
                        ins=[cc_in[t][:].opt()], outs=[cc_out[t][:].opt()],
                    )

            all_reduce(0)

            # ---------------- routing iterations ----------------
            with (
                tc.tile_pool(name="rout", bufs=1) as rout,
                tc.tile_pool(name="ring", bufs=2) as ring,
                tc.tile_pool(name="tring", bufs=2) as tring,
                tc.tile_pool(name="ps_b", bufs=3, space="PSUM") as ps_b,
            ):
                OB2P = B * OC * NCAP     # obd2 pitch (8192)
                OTP = B * OC             # o_tmp pitch (256)
                o_tmp = rout.tile([128, B, OC], f32, tag="o_tmp")
                obd2 = rout.tile([128, OB2P], bf16, tag="obd2")
                blog = rout.tile([128, B, 2, NCAP], f32, tag="blog")
                c_sb = rout.tile([128, B, 2, NCAP], bf16, tag="c_sb")
                sthi = rout.tile([NCAP, 8, 256], bf16, tag="sthi")
                stlo = rout.tile([NCAP, 8, 256], bf16, tag="stlo")

                # zero once; the mask copies overwrite the same cols every iter
                ms2 = nc.gpsimd.memset(obd2[:], 0.0)
                memsets = [ms2]

                for t in range(3):
                    final = t == 2
                    ji = t > 0  # cc[0] is [b,(i,j)]; cc[1:] are [j,(b,i)]
                    # ---- load global o, normalize (l2 for t<2, squash at t=2)
                    if not ji:
                        nc.sync.dma_start(onrm[:], cc_out[t][:])
                    else:
                        # onrm[b, j*32+i] = cc_out[j, b*32+i]
                        dstL = _set_dims(
                            onrm[:], [[O, B], [NCAP, DIM], [1, NCAP]]
                        )
                        srcL = _set_dims(
                            cc_out[t][:], [[NCAP, B], [O, DIM], [1, NCAP]]
                        )
                        nc.sync.dma_start(dstL, srcL)
                    nc.scalar.square(onl[:], onrm[:])
                    if not ji:
                        red_in = onl[:].rearrange("b (i j) -> b i j", i=NCAP)
                    else:
                        red_in = _set_dims(
                            onl[:], [[O, B], [1, NCAP], [NCAP, DIM]]
                        )
                    nc.vector.tensor_reduce(s2[:], red_in, axis=AX.X, op=OP.add)
                    if not final:
                        nc.vector.tensor_scalar_max(s2b[:], s2[:], EPS_L2)
                        nc.scalar.sqrt(s2c[:], s2b[:])
                        nc.vector.reciprocal(rinv[:], s2c[:])
                    else:
                        # squash scale = s2 / ((1+s2) * sqrt(s2+eps))
                        nc.vector.tensor_scalar_add(s2b[:], s2[:], EPS_K)
                        nc.scalar.sqrt(s2b[:], s2b[:])
                        nc.vector.tensor_scalar_add(s2c[:], s2[:], 1.0)
                        nc.vector.tensor_mul(s2c[:], s2c[:], s2b[:])
                        nc.vector.reciprocal(s2b[:], s2c[:])
                        nc.vector.tensor_mul(rinv[:], s2b[:], s2[:])
                    if not ji:
                        sc_b = _set_dims(
                            rinv[:], [[NCAP, B], [1, NCAP], [0, DIM]]
                        )
                        nc.vector.tensor_mul(
                            onl[:].rearrange("b (i j) -> b i j", i=NCAP),
                            onrm[:].rearrange("b (i j) -> b i j", i=NCAP),
                            sc_b,
                        )
                    else:
                        # loops (b, j, i); onl written in (i,j) order at t=2
                        sc_b = _set_dims(
                            rinv[:], [[NCAP, B], [0, DIM], [1, NCAP]]
                        )
                        in_ji = _set_dims(
                            onrm[:], [[O, B], [NCAP, DIM], [1, NCAP]]
                        )
                        if final:
                            out_v = _set_dims(
                                onl[:], [[O, B], [1, DIM], [DIM, NCAP]]
                            )
                        else:
                            out_v = _set_dims(
                                onl[:], [[O, B], [NCAP, DIM], [1, NCAP]]
                            )
                        nc.vector.tensor_mul(out_v, in_ji, sc_b)
                    if final:
                        nc.sync.dma_start(out_d[:], onl[:])
                        break

                    # ---- scatter normalized o into block-diag (cast to bf16)
                    # bounce through DRAM: SBUF DMA APs need partition dim first
                    if not ji:
                        nc.sync.dma_start(o_dram[:], onl[:])
                    else:
                        # o_dram[j, b*32+i] = onl[b, j*32+i]
                        dstJ = _set_dims(
                            o_dram[:], [[NCAP, B], [O, DIM], [1, NCAP]]
                        )
                        srcJ = _set_dims(
                            onl[:], [[O, B], [NCAP, DIM], [1, NCAP]]
                        )
                        nc.sync.dma_start(dstJ, srcJ)
                    # o_tmp[p=i4*32+j, b*8+oc] = o(b, oc*4+i4, j); one DMA per i4
                    for i4 in range(4):
                        if not ji:
                            srcd = _set_dims(
                                o_dram[:], [[1, DIM], [128, 256]],
                                offset=i4 * DIM,
                            )
                        else:
                            srcd = _set_dims(
                                o_dram[:], [[O, DIM], [4, 256]], offset=i4
                            )
                        dstd = _set_dims(
                            o_tmp[:], [[OTP, DIM], [1, 256]],
                            offset=(32 * i4) * OTP,
                        )
                        nc.sync.dma_start(dstd, srcd)
                    # masked strided copies into obd2:
                    # obd2[32m+j, b*256 + oc*36 + m] = o_tmp[32m+j, b*8+oc]
                    for m in range(4):
                        src_e = _set_dims(
                            o_tmp[:], [[OTP, DIM], [OC, B], [1, OC]],
                            offset=(32 * m) * OTP,
                        )
                        dst_e = _set_dims(
                            obd2[:],
                            [[OB2P, DIM], [OC * NCAP, B], [NCAP + 4, OC]],
                            offset=(32 * m) * OB2P + m,
                        )
                        if m % 2 == 0:
                            ec = nc.vector.tensor_copy(dst_e, src_e)
                        else:
                            ec = nc.scalar.copy(dst_e, src_e)
                        if t == 0:
                            for ms in memsets:
                                add_dep_helper(
                                    ec.ins, ms.ins, sync=True,
                                    reason="mask copy after memset",
                                )

                    # ---- b-update: per b, 8 oc-matmuls accumulate [32, 256]
                    for b in range(B):
                        psb = ps_b.tile([NCAP, 256], f32, tag="psb")
                        for oc in range(OC):
                            lhs = _set_dims(
                                obd2[:], [[OB2P, 128], [1, NCAP]],
                                offset=b * OC * NCAP + oc * NCAP,
                            )
                            nc.tensor.matmul(
                                psb[:], lhs, u_hat[:, b, oc, :],
                                start=(oc == 0), stop=(oc == OC - 1),
                            )
                        bg, b8 = b >> 3, b & 7
                        nc.scalar.copy(sthi[:, b8, :], psb[:])
                        nc.vector.tensor_sub(stlo[:, b8, :], psb[:], sthi[:, b8, :])
                        if b8 == 7:
                            # transpose [32 i, 2048 (b8,n)] -> [128 nl, (b8,nh), 32 i]
                            thi = tring.tile([128, 16, NCAP], bf16, tag="thi")
                            tlo = tring.tile([128, 16, NCAP], bf16, tag="tlo")
                            nc.sync.dma_start_transpose(
                                thi[:], sthi[:].rearrange("p a n -> p (a n)")
                            )
                            nc.sync.dma_start_transpose(
                                tlo[:], stlo[:].rearrange("p a n -> p (a n)")
                            )
                            nc.vector.tensor_add(
                                blog[:, bg * 8:(bg + 1) * 8, :, :],
                                thi[:].rearrange("p (b h) i -> p b h i", b=8),
                                tlo[:].rearrange("p (b h) i -> p b h i", b=8),
                            )

                    # ---- softmax over i on blog [p=nl, (b, nh, i)]
                    nc.vector.tensor_reduce(mx[:], blog[:], axis=AX.X, op=OP.max)
                    mxb = _set_dims(
                        mx[:], [[2 * B, 128], [1, 2 * B], [0, NCAP]]
                    )
                    blog3 = blog[:].rearrange("p b h i -> p (b h) i")
                    nc.vector.tensor_sub(blog3, blog3, mxb)
                    nc.scalar.activation(blog[:], blog[:], AF.Exp)
                    nc.vector.tensor_reduce(sm[:], blog[:], axis=AX.X, op=OP.add)
                    nc.vector.reciprocal(smr[:], sm[:])
                    smb = _set_dims(
                        smr[:], [[2 * B, 128], [1, 2 * B], [0, NCAP]]
                    )
                    nc.vector.tensor_mul(
                        c_sb[:].rearrange("p b h i -> p (b h) i"), blog3, smb
                    )

                    # ---- o-pass: xbar-transpose u_hat per 2b, matmul with c
                    for b in range(B):
                        cg = b & 3
                        if cg == 0:
                            pso = ps_main.tile([128, 1024], f32, tag="ps")
                        if b % 2 == 0:
                            uht = ring.tile([128, 32, 128], bf16, tag="uht")
                            nc.sync.dma_start_transpose(
                                uht[:],
                                u_hat[:, b:b + 2, :, :].rearrange(
                                    "p b a n -> p (b a n)"
                                ),
                            )
                        b1 = b & 1
                        for nh in range(2):
                            lhs = c_sb[:, b, nh, :]
                            for oh in range(2):
                                rhs = _set_dims(
                                    uht[:],
                                    [[32 * 128, 128], [256, 4], [1, 128]],
                                    offset=(16 * b1 + 8 * oh + nh) * 128,
                                )
                                nc.tensor.matmul(
                                    pso[32 * cg:32 * cg + 32,
                                        oh * 512:(oh + 1) * 512],
                                    lhs, rhs,
                                    start=(nh == 0), stop=(nh == 1),
                                    tile_position=(0, 32 * cg),
                                    skip_group_check=True,
                                )
                        if cg == 3:
                            # 32x32 block transpose; diag becomes stride-33 cols
                            nc.vector.transpose(tr_scr[:], pso[:])
                            for c2 in range(4):
                                bb = b - 3 + c2
                                diag = _set_dims(
                                    tr_scr[:], [[1024, 32], [33, DIM]],
                                    offset=(32 * c2) * 1024,
                                )
                                if c2 % 2 == 0:
                                    nc.scalar.copy(o_acc[:, bb, :], diag)
                                else:
                                    nc.vector.tensor_copy(o_acc[:, bb, :], diag)

                    # o_acc [j, b, i] -> cc_in[t+1] (ji layout, same shape)
                    nc.sync.dma_start(cc_in[t + 1][:], o_acc[:])
                    all_reduce(t + 1)

    nc.compile()
    return nc


def host_prep(u_vecs, W, core):
    ns = slice(core * NL, (core + 1) * NL)
    Wc = np.asarray(W[ns], dtype=np.float32)             # [NL, 16, 1024]
    uc = np.asarray(u_vecs[:, ns, :], dtype=np.float32)  # [B, NL, 16]
    bf = ml_dtypes.bfloat16

    wl = (
        Wc.reshape(G, 8, KD, OC, 128)
        .transpose(0, 3, 1, 2, 4)
        .reshape(G, OC, 128, 128)
        .astype(bf)
    )
    tmp = uc.transpose(1, 2, 0).reshape(G, 8, KD, B)     # [g, n8, k, b]
    ubd = np.zeros((G, 8, KD, B, 8), dtype=np.float32)
    for n8 in range(8):
        ubd[:, n8, :, :, n8] = tmp[:, n8]
    ubd = ubd.reshape(G, 128, B * 8).astype(bf)
    upl = tmp.reshape(G, 128, B).astype(bf)
    return {"wl": wl, "ubd": ubd, "upl": upl}


def kernel(u_vecs, W):
    from concourse import bass_utils

    if "prog" not in _PROG:
        _PROG["prog"] = build_program(NUM_CORES)
    nc = _PROG["prog"]
    in_maps = [host_prep(u_vecs, W, c) for c in range(NUM_CORES)]
    res = bass_utils.run_bass_kernel_spmd(
        nc, in_maps, core_ids=list(range(NUM_CORES))
    )
    out = np.asarray(res.results[0]["out"], dtype=np.float32)
    return out.reshape(B, NCAP, DIM)



# revision 23
# speedup vs baseline: 1.2921x; 1.2921x over previous
"""Trainium2 Bass kernel for CapsuleLayer dynamic routing (8-core SPMD).

Strategy (v2):
- Shard the 2048 input capsules (n) across 8 cores (W is the big input; each
  core loads only its 1/8 slice). Routing needs a global o => 3 AllReduces of
  the small [32,1024] o tensor.
- Build u_hat once on the PE (block-diag u trick: 8 n's x 16 k = full 128-row
  contraction), then keep it resident in TWO layouts:
    (a)  [128 p=(i%4,j),  cols (b, oc, s)]  fp8 hi  -- streamed by the b-update
    (T)  [128 p=t,        cols (b, oc, ij, h)] fp8 hi -- streamed by the o-pass
  plus an fp8 "lo" residual (u_hat - hi) spilled to DRAM in (a)-layout and
  XBAR-transposed into (T)-layout chunks just in time for each o-pass.
  s = 2t+h with n = h*128+t (n-interleaved storage) makes the uint16-pair XBAR
  transpose of the fp8 bytes land (n, n+128) pairs exactly where the fp8
  DoubleRow matmul wants its K=256 pair structure.
- All routing contractions run as fp8e4 DoubleRow (2x PE throughput):
    b-update: stationary (o_hi, o_lo) pairs x moving u_hi (dup'd via stride-0)
    o-pass:   stationary c pairs x moving (T)hi / (T)lo chunks
  Accuracy (numpy model of this exact recipe): rel ~ 1.3e-2 < 2e-2.
"""
import sys

sys.path.insert(0, "/opt/trn_rl_repo")

import numpy as np
import ml_dtypes

B = 32            # batch
N_TOTAL = 2048    # input capsules
KD = 16           # input capsule dim
NCAP = 32         # output capsules (i)
DIM = 32          # output capsule dim (j)
O = NCAP * DIM    # 1024
NUM_CORES = 8
NL = N_TOTAL // NUM_CORES   # 256 local n
G = NL // 8                 # 32 groups of 8 n
G4 = 8                      # 8 chunks of 4 groups
OC = O // 128               # 8 o-chunks
UA_P = B * OC * NL          # 65536: u_a / ulo / u_t per-partition cols
OB_P = B * 576              # obd2 pitch
EPS_K = 1e-7
EPS_L2 = 1e-12

_PROG = {}
DEBUG = False
STAGES = 3  # routing iterations to emit (3 = full kernel); for perf bisection


def _set_dims(ap, dims, offset=None):
    c = ap.copy()
    v = c.ap
    while len(v) > len(dims):
        v.pop()
    while len(v) < len(dims):
        v.insert(0, [0, 1])
    for k, d in enumerate(dims):
        v[k] = list(d)
    if offset is not None:
        c.offset = offset
    return c


def build_program(num_cores):
    import concourse.bass as bass
    import concourse.mybir as mybir
    from concourse import bacc, tile
    from concourse.tile import add_dep_helper

    f32 = mybir.dt.float32
    bf16 = mybir.dt.bfloat16
    e4 = mybir.dt.float8e4
    AX = mybir.AxisListType
    OP = mybir.AluOpType
    AF = mybir.ActivationFunctionType
    DR = mybir.MatmulPerfMode.DoubleRow

    nc = bacc.Bacc("TRN2", target_bir_lowering=False, num_devices=num_cores)
    rg = [list(range(num_cores))]

    wl_d = nc.dram_tensor("wl", [G4, 128, 4096], bf16, kind="ExternalInput")
    ubd_d = nc.dram_tensor("ubd", [G4, 128, 1024], bf16, kind="ExternalInput")
    upl_d = nc.dram_tensor("upl", [128, G * B], bf16, kind="ExternalInput")
    out_d = nc.dram_tensor("out", [B, O], f32, kind="ExternalOutput")

    with tile.TileContext(nc) as tc:
        with (
            tc.tile_pool(name="pers", bufs=1) as pers,
            tc.tile_pool(name="dram", bufs=1, space="DRAM") as dram,
            tc.tile_pool(name="ps_o0", bufs=1, space="PSUM") as ps_o0,
            tc.tile_pool(name="small", bufs=1) as small,
        ):
            u_a = pers.tile([128, UA_P], e4, tag="u_a")       # (a)-layout hi
            o_acc = pers.tile([DIM, B, NCAP], f32, tag="o_acc")
            o0_sb = pers.tile([128, OC * B], f32, tag="o0_sb")

            onrm = small.tile([B, O], f32, tag="onrm")
            onl = small.tile([B, O], f32, tag="onl")
            s2 = small.tile([B, NCAP], f32, tag="s2")
            s2b = small.tile([B, NCAP], f32, tag="s2b")
            s2c = small.tile([B, NCAP], f32, tag="s2c")
            rinv = small.tile([B, NCAP], f32, tag="rinv")

            ulo_d = dram.tile([128, UA_P], e4, tag="ulo_d", name="ulo_d")
            o_dram = dram.tile([B, O], f32, tag="o_dram", name="o_dram")
            cc_in = [
                dram.tile([B, O], f32, tag=f"cc_in{t}", name=f"cc_in{t}")
                for t in range(3)
            ]
            cc_out = [
                dram.tile([B, O], f32, tag=f"cc_out{t}", name=f"cc_out{t}")
                for t in range(3)
            ]
            if DEBUG:
                dbg_ua = nc.dram_tensor("dbg_ua", [128, UA_P], e4,
                                        kind="ExternalOutput")
                dbg_ut = nc.dram_tensor("dbg_ut", [128, UA_P], e4,
                                        kind="ExternalOutput")
                dbg_ulo = nc.dram_tensor("dbg_ulo", [128, UA_P], e4,
                                         kind="ExternalOutput")
                dbg_obd = nc.dram_tensor("dbg_obd", [128, OB_P], e4,
                                         kind="ExternalOutput")
                dbg_ch = [
                    nc.dram_tensor(f"dbg_ch{t}", [128, B * 2 * NCAP], e4,
                                   kind="ExternalOutput") for t in range(2)
                ]
                dbg_st = [
                    nc.dram_tensor(f"dbg_st{t}", [4, NCAP, 8 * 256], bf16,
                                   kind="ExternalOutput") for t in range(2)
                ]
                dbg_o0p = nc.dram_tensor("dbg_o0p", [B, O], f32,
                                         kind="ExternalOutput")
                dbg_o0g = nc.dram_tensor("dbg_o0g", [B, O], f32,
                                         kind="ExternalOutput")

            # ---------------- Phase A: build u_hat hi/lo + o0 ----------------
            with (
                tc.tile_pool(name="bpool", bufs=1) as bpool,
                tc.tile_pool(name="wpool", bufs=2) as wpool,
                tc.tile_pool(name="ubpool", bufs=2) as ubpool,
                tc.tile_pool(name="ulopool", bufs=1) as ulopool,
                tc.tile_pool(name="ps_bld", bufs=3, space="PSUM") as ps_bld,
            ):
                upl_sb = bpool.tile([128, G * B], bf16, tag="upl_sb")
                nc.sync.dma_start(upl_sb[:], upl_d[:])
                ulo_sb = ulopool.tile([128, UA_P], e4, tag="ulo_sb")
                o0ps = ps_o0.tile([128, OC * B], f32, tag="o0ps")

                for g4 in range(G4):
                    wl_sb = wpool.tile([128, 4096], bf16, tag="wl_sb")
                    nc.sync.dma_start(wl_sb[:], wl_d[g4, :, :])
                    ubd_sb = ubpool.tile([128, 1024], bf16, tag="ubd_sb")
                    nc.sync.dma_start(ubd_sb[:], ubd_d[g4, :, :])
                    base_s = 64 * g4 if g4 < 4 else 64 * g4 - 255
                    for oc in range(OC):
                        ps = ps_bld.tile([128, 1024], f32, tag="psb_a")
                        for gh in range(4):
                            wsl = wl_sb[:, (oc * 4 + gh) * 128:(oc * 4 + gh + 1) * 128]
                            nc.tensor.matmul(
                                ps[:, gh * 256:(gh + 1) * 256],
                                wsl, ubd_sb[:, gh * 256:(gh + 1) * 256],
                                start=True, stop=True,
                                skip_group_check=True,
                            )
                            g = g4 * 4 + gh
                            # one start for the whole tile: ZERO_REGION covers
                            # all oc sub-regions; their first write zero-clears
                            nc.tensor.matmul(
                                o0ps[:, oc * B:(oc + 1) * B],
                                wsl, upl_sb[:, g * B:(g + 1) * B],
                                start=(g4 == 0 and oc == 0 and gh == 0),
                                stop=(g4 == G4 - 1 and oc == OC - 1 and gh == 3),
                                skip_group_check=True,
                            )
                        src = ps[:].rearrange("p (g b n) -> p g b n", g=4, b=B)
                        dims = [[UA_P, 128], [16, 4], [OC * NL, B], [2, 8]]
                        off = oc * NL + base_s
                        dst_hi = _set_dims(u_a[:], dims, offset=off)
                        dst_lo = _set_dims(ulo_sb[:], dims, offset=off)
                        idx = g4 * 8 + oc
                        if idx % 8 == 7:
                            nc.gpsimd.tensor_copy(dst_hi, src)
                        else:
                            nc.scalar.copy(dst_hi, src)
                        if idx % 8 == 3:
                            nc.gpsimd.tensor_sub(dst_lo, src, dst_hi)
                        else:
                            nc.vector.tensor_sub(dst_lo, src, dst_hi)
                nc.scalar.copy(o0_sb[:], o0ps[:])
                # spill lo residual ((a)-layout) to DRAM
                nc.sync.dma_start(ulo_d[:], ulo_sb[:])

            # o0 partial -> cc_in0[b, oc*128+p] = o0_sb[p, oc*B+b]
            for oc in range(OC):
                src = o0_sb[:, oc * B:(oc + 1) * B]
                dst = _set_dims(cc_in[0][:], [[1, 128], [O, B]], offset=oc * 128)
                nc.sync.dma_start(dst, src)

            def all_reduce(t):
                if num_cores == 1:
                    nc.gpsimd.dma_start(cc_out[t][:], cc_in[t][:])
                else:
                    nc.gpsimd.collective_compute(
                        "AllReduce", OP.add, replica_groups=rg,
                        ins=[cc_in[t][:].opt()], outs=[cc_out[t][:].opt()],
                    )

            all_reduce(0)

            # ---------------- routing ----------------
            with (
                tc.tile_pool(name="rout", bufs=1) as rout,
                tc.tile_pool(name="sring", bufs=2) as sring,
                tc.tile_pool(name="tring", bufs=2) as tring,
                tc.tile_pool(name="loring", bufs=2) as loring,
                tc.tile_pool(name="ps_main", bufs=2, space="PSUM") as ps_main,
                tc.tile_pool(name="ps_b", bufs=3, space="PSUM") as ps_b,
            ):
                u_t = rout.tile([128, UA_P], e4, tag="u_t")   # (T)-layout hi
                obd2 = rout.tile([128, OB_P], e4, tag="obd2")
                c_hi = rout.tile([128, B * 2 * NCAP], e4, tag="c_hi")
                c_lo = rout.tile([128, B * 2 * NCAP], e4, tag="c_lo")
                o_tmp = rout.tile([128, B * OC], f32, tag="o_tmp")
                ohi_t = rout.tile([128, B * OC], e4, tag="ohi_t")
                olo_t = rout.tile([128, B * OC], e4, tag="olo_t")
                tr_scr = rout.tile([128, 1024], f32, tag="tr_scr")
                mx = rout.tile([128, 16], f32, tag="mx")
                sm = rout.tile([128, 16], f32, tag="sm")
                smr = rout.tile([128, 16], f32, tag="smr")

                # XBAR (a)hi -> (T)hi: uint16 view pairs (s,s+1) = (n_t, n_{t+128})
                ua16 = u_a[:].bitcast(bf16)
                ut16 = u_t[:].bitcast(bf16)
                for b4 in range(B // 4):
                    src = _set_dims(ua16, [[UA_P // 2, 128], [1, 4096]],
                                    offset=b4 * 4096)
                    dst = _set_dims(ut16,
                                    [[UA_P // 2, 128], [128, 32], [1, 128]],
                                    offset=b4 * 4096)
                    nc.sync.dma_start_transpose(dst, src)

                ms = nc.gpsimd.memset(obd2[:], 0.0)
                memsets = [ms]
                if DEBUG:
                    nc.sync.dma_start(dbg_ua[:], u_a[:])
                    nc.sync.dma_start(dbg_ut[:], u_t[:])
                    nc.gpsimd.dma_start(dbg_ulo[:], ulo_d[:])
                    nc.gpsimd.dma_start(dbg_o0p[:], cc_in[0][:])
                    nc.gpsimd.dma_start(dbg_o0g[:], cc_out[0][:])

                for t in range(STAGES):
                    final = t == 2
                    ji = t > 0  # cc[0] is [b,(i,j)]; cc[1:] are [j,(b,i)]
                    # ---- load global o, normalize (l2 for t<2, squash at t=2)
                    if not ji:
                        nc.sync.dma_start(onrm[:], cc_out[t][:])
                    else:
                        dstL = _set_dims(
                            onrm[:], [[O, B], [NCAP, DIM], [1, NCAP]]
                        )
                        srcL = _set_dims(
                            cc_out[t][:], [[NCAP, B], [O, DIM], [1, NCAP]]
                        )
                        nc.sync.dma_start(dstL, srcL)
                    nc.scalar.square(onl[:], onrm[:])
                    if not ji:
                        red_in = onl[:].rearrange("b (i j) -> b i j", i=NCAP)
                    else:
                        red_in = _set_dims(
                            onl[:], [[O, B], [1, NCAP], [NCAP, DIM]]
                        )
                    nc.vector.tensor_reduce(s2[:], red_in, axis=AX.X, op=OP.add)
                    if not final:
                        nc.vector.tensor_scalar_max(s2b[:], s2[:], EPS_L2)
                        nc.scalar.sqrt(s2c[:], s2b[:])
                        nc.vector.reciprocal(rinv[:], s2c[:])
                    else:
                        nc.vector.tensor_scalar_add(s2b[:], s2[:], EPS_K)
                        nc.scalar.sqrt(s2b[:], s2b[:])
                        nc.vector.tensor_scalar_add(s2c[:], s2[:], 1.0)
                        nc.vector.tensor_mul(s2c[:], s2c[:], s2b[:])
                        nc.vector.reciprocal(s2b[:], s2c[:])
                        nc.vector.tensor_mul(rinv[:], s2b[:], s2[:])
                    if not ji:
                        sc_b = _set_dims(
                            rinv[:], [[NCAP, B], [1, NCAP], [0, DIM]]
                        )
                        nc.vector.tensor_mul(
                            onl[:].rearrange("b (i j) -> b i j", i=NCAP),
                            onrm[:].rearrange("b (i j) -> b i j", i=NCAP),
                            sc_b,
                        )
                    else:
                        sc_b = _set_dims(
                            rinv[:], [[NCAP, B], [0, DIM], [1, NCAP]]
                        )
                        in_ji = _set_dims(
                            onrm[:], [[O, B], [NCAP, DIM], [1, NCAP]]
                        )
                        if final:
                            out_v = _set_dims(
                                onl[:], [[O, B], [1, DIM], [DIM, NCAP]]
                            )
                        else:
                            out_v = _set_dims(
                                onl[:], [[O, B], [NCAP, DIM], [1, NCAP]]
                            )
                        nc.vector.tensor_mul(out_v, in_ji, sc_b)
                    if final:
                        nc.sync.dma_start(out_d[:], onl[:])
                        break

                    # ---- scatter normalized o into block-diag hi/lo fp8
                    if not ji:
                        nc.sync.dma_start(o_dram[:], onl[:])
                    else:
                        dstJ = _set_dims(
                            o_dram[:], [[NCAP, B], [O, DIM], [1, NCAP]]
                        )
                        srcJ = _set_dims(
                            onl[:], [[O, B], [NCAP, DIM], [1, NCAP]]
                        )
                        nc.sync.dma_start(dstJ, srcJ)
                    # o_tmp[p=i4*32+j, b*8+oc] = o(b, oc*4+i4, j)
                    for i4 in range(4):
                        if not ji:
                            srcd = _set_dims(
                                o_dram[:], [[1, DIM], [128, 256]],
                                offset=i4 * DIM,
                            )
                        else:
                            srcd = _set_dims(
                                o_dram[:], [[O, DIM], [4, 256]], offset=i4
                            )
                        dstd = _set_dims(
                            o_tmp[:], [[B * OC, DIM], [1, 256]],
                            offset=(32 * i4) * B * OC,
                        )
                        nc.sync.dma_start(dstd, srcd)
                    nc.scalar.copy(ohi_t[:], o_tmp[:])
                    nc.vector.tensor_sub(olo_t[:], o_tmp[:], ohi_t[:])
                    # masked strided copies into obd2 (s-plane offset 288):
                    # obd2[32m+j, b*576 + s*288 + oc*36 + m] = o_s[32m+j, b*8+oc]
                    for m in range(4):
                        for s, stile in ((0, ohi_t), (1, olo_t)):
                            src_e = _set_dims(
                                stile[:], [[B * OC, DIM], [OC, B], [1, OC]],
                                offset=(32 * m) * B * OC,
                            )
                            dst_e = _set_dims(
                                obd2[:],
                                [[OB_P, DIM], [576, B], [36, OC]],
                                offset=(32 * m) * OB_P + s * 288 + m,
                            )
                            if (m + s) % 2 == 0:
                                ec = nc.vector.tensor_copy(dst_e, src_e)
                            else:
                                ec = nc.scalar.copy(dst_e, src_e)
                            if t == 0:
                                for msi in memsets:
                                    add_dep_helper(
                                        ec.ins, msi.ins, sync=True,
                                        reason="mask copy after memset",
                                    )

                    # ---- b-update (fp8 DoubleRow): psb[i', s] logits
                    for b in range(B):
                        psb = ps_b.tile([NCAP, 256], f32, tag="psb")
                        for oc in range(OC):
                            lhs = _set_dims(
                                obd2[:], [[OB_P, 128], [288, 2], [1, 32]],
                                offset=b * 576 + oc * 32,
                            )
                            rhs = _set_dims(
                                u_a[:], [[UA_P, 128], [0, 2], [1, 256]],
                                offset=b * 2048 + oc * 256,
                            )
                            nc.tensor.matmul(
                                psb[:], lhs, rhs,
                                start=(oc == 0), stop=(oc == OC - 1),
                                perf_mode=DR,
                            )
                        bg, b8 = b >> 3, b & 7
                        if b8 == 0:
                            sthi = sring.tile([NCAP, 8, 256], bf16, tag="sthi")
                        # reorder cols s=(2t+h) -> (h, t) while staging to bf16
                        srcp = _set_dims(psb[:], [[256, NCAP], [1, 2], [2, 128]])
                        dstp = _set_dims(
                            sthi[:], [[8 * 256, NCAP], [128, 2], [1, 128]],
                            offset=b8 * 256,
                        )
                        if b8 % 2 == 0:
                            nc.scalar.copy(dstp, srcp)
                        else:
                            nc.gpsimd.tensor_copy(dstp, srcp)
                        if DEBUG and b8 == 7 and t == 0:
                            nc.sync.dma_start(dbg_st[0][bg, :, :], sthi[:])
                        if b8 == 7:
                            # transpose [32 i, (b8, h, t)] -> [128 t, (b8, h), i]
                            thi = tring.tile([128, 16, NCAP], bf16, tag="thi")
                            nc.sync.dma_start_transpose(
                                thi[:], sthi[:].rearrange("p a n -> p (a n)")
                            )
                            cexp = tring.tile([128, 16, NCAP], f32, tag="cexp")
                            # softmax over i for this b-group
                            nc.vector.tensor_reduce(
                                mx[:], thi[:], axis=AX.X, op=OP.max
                            )
                            mxb = _set_dims(
                                mx[:], [[16, 128], [1, 16], [0, NCAP]]
                            )
                            nc.vector.tensor_sub(cexp[:], thi[:], mxb)
                            nc.scalar.activation(cexp[:], cexp[:], AF.Exp)
                            nc.vector.tensor_reduce(
                                sm[:], cexp[:], axis=AX.X, op=OP.add
                            )
                            nc.vector.reciprocal(smr[:], sm[:])
                            smb = _set_dims(
                                smr[:], [[16, 128], [1, 16], [0, NCAP]]
                            )
                            chs = _set_dims(
                                c_hi[:], [[B * 2 * NCAP, 128], [1, 16 * NCAP]],
                                offset=bg * 16 * NCAP,
                            ).rearrange("p (a n) -> p a n", a=16)
                            if t == 0:
                                nc.vector.tensor_mul(chs, cexp[:], smb)
                            else:
                                nc.vector.tensor_mul(cexp[:], cexp[:], smb)
                                nc.scalar.copy(chs, cexp[:])
                                cls = _set_dims(
                                    c_lo[:],
                                    [[B * 2 * NCAP, 128], [1, 16 * NCAP]],
                                    offset=bg * 16 * NCAP,
                                ).rearrange("p (a n) -> p a n", a=16)
                                nc.vector.tensor_sub(cls, cexp[:], chs)

                    if DEBUG and t == 0:
                        nc.sync.dma_start(dbg_obd[:], obd2[:])
                    if DEBUG:
                        nc.sync.dma_start(dbg_ch[t][:], c_hi[:])
                    # ---- o-pass (fp8 DoubleRow): hi + lo (+ c_lo at t=1)
                    ulo16 = ulo_d[:].bitcast(bf16)
                    for b in range(B):
                        cg = b & 3
                        if cg == 0:
                            pso = ps_main.tile([128, 1024], f32, tag="ps")
                            if t == 1:
                                tlo = loring.tile([128, 4, OC, 128, 2], e4,
                                                  tag="tlo")
                                tlo16 = tlo[:].bitcast(bf16)
                                src = _set_dims(
                                    ulo16, [[UA_P // 2, 128], [1, 4096]],
                                    offset=(b >> 2) * 4096,
                                )
                                dst = _set_dims(
                                    tlo16,
                                    [[4 * OC * 128, 128], [128, 32], [1, 128]]
                                )
                                nc.sync.dma_start_transpose(dst, src)
                        ch_b = _set_dims(
                            c_hi[:], [[B * 2 * NCAP, 128], [NCAP, 2], [1, NCAP]],
                            offset=b * 2 * NCAP,
                        )
                        cl_b = _set_dims(
                            c_lo[:], [[B * 2 * NCAP, 128], [NCAP, 2], [1, NCAP]],
                            offset=b * 2 * NCAP,
                        )
                        for oc in range(OC):
                            out_ap = pso[32 * cg:32 * cg + 32,
                                         oc * 128:(oc + 1) * 128]
                            rhs_hi = _set_dims(
                                u_t[:], [[UA_P, 128], [1, 2], [2, 128]],
                                offset=b * 2048 + oc * 256,
                            )
                            nc.tensor.matmul(
                                out_ap, ch_b, rhs_hi,
                                start=True, stop=(t == 0), perf_mode=DR,
                                tile_position=(0, 32 * cg),
                                skip_group_check=True,
                            )
                            if t == 1:
                                rhs_lo = _set_dims(
                                    tlo[:],
                                    [[4 * OC * 256, 128], [1, 2], [2, 128]],
                                    offset=cg * OC * 256 + oc * 256,
                                )
                                nc.tensor.matmul(
                                    out_ap, ch_b, rhs_lo,
                                    start=False, stop=False, perf_mode=DR,
                                    tile_position=(0, 32 * cg),
                                    skip_group_check=True,
                                )
                                nc.tensor.matmul(
                                    out_ap, cl_b, rhs_hi,
                                    start=False, stop=True, perf_mode=DR,
                                    tile_position=(0, 32 * cg),
                                    skip_group_check=True,
                                )
                        if cg == 3:
                            # 32x32 block transpose; diag becomes stride-33 cols
                            nc.vector.transpose(tr_scr[:], pso[:])
                            for c2 in range(4):
                                bb = b - 3 + c2
                                diag = _set_dims(
                                    tr_scr[:], [[1024, 32], [33, DIM]],
                                    offset=(32 * c2) * 1024,
                                )
                                if c2 % 2 == 0:
                                    nc.scalar.copy(o_acc[:, bb, :], diag)
                                else:
                                    nc.vector.tensor_copy(o_acc[:, bb, :], diag)

                    # o_acc [j, b, i] -> cc_in[t+1] (ji layout)
                    nc.sync.dma_start(cc_in[t + 1][:], o_acc[:])
                    all_reduce(t + 1)

    nc.compile()
    return nc


def host_prep(u_vecs, W, core):
    ns = slice(core * NL, (core + 1) * NL)
    Wc = np.asarray(W[ns], dtype=np.float32)             # [NL, 16, 1024]
    uc = np.asarray(u_vecs[:, ns, :], dtype=np.float32)  # [B, NL, 16]
    bf = ml_dtypes.bfloat16

    # wl [g4, (n8, k), (oc, gh, 128)]
    wl = (
        Wc.reshape(G4, 4, 8, KD, OC, 128)   # g4, gh, n8, k, oc, oco
        .transpose(0, 2, 3, 4, 1, 5)        # g4, n8, k, oc, gh, oco
        .reshape(G4, 128, 4096)
        .astype(bf)
    )
    tmp = uc.transpose(1, 2, 0).reshape(G, 8, KD, B)     # [g, n8, k, b]
    ubd = np.zeros((G, 8, KD, B, 8), dtype=np.float32)
    for n8 in range(8):
        ubd[:, n8, :, :, n8] = tmp[:, n8]
    # [G, 128, 256] -> [g4, (n8,k), (gh, b, n8')]
    ubd = (
        ubd.reshape(G4, 4, 128, 256)
        .transpose(0, 2, 1, 3)
        .reshape(G4, 128, 1024)
        .astype(bf)
    )
    upl = (
        tmp.reshape(G, 128, B).transpose(1, 0, 2).reshape(128, G * B).astype(bf)
    )
    return {"wl": wl, "ubd": ubd, "upl": upl}


def kernel(u_vecs, W):
    from concourse import bass_utils

    if "prog" not in _PROG:
        _PROG["prog"] = build_program(NUM_CORES)
    nc = _PROG["prog"]
    in_maps = [host_prep(u_vecs, W, c) for c in range(NUM_CORES)]
    res = bass_utils.run_bass_kernel_spmd(
        nc, in_maps, core_ids=list(range(NUM_CORES))
    )
    out = np.asarray(res.results[0]["out"], dtype=np.float32)
    return out.reshape(B, NCAP, DIM)
